# revision 30
# baseline (speedup 1.0000x reference)
"""MeanShift retrieval-KNN loss kernel for 8 Trainium2 NeuronCores — v3.

Reference computation (B=4096, K=32768, DIM=512, TOPK=5):
    query  = l2norm(query_raw); target = l2norm(target_raw)
    qbank  = l2norm(queue); qbank[0:B] = target
    dist_t = 2 - 2 * target @ qbank.T ; dist_q = 2 - 2 * query @ qbank.T
    idx    = top5 smallest dist_t per row
    loss   = mean_b( sum_j dist_q[b, idx[b,j]] / 5 )

Sharding: queue K axis split across 8 cores (4096 rows each); core 0's
shard is target_raw (the reference overwrites bank rows 0:B, and raw
queue rows 0:B are never read).

v3 design (vs v2):
  * t/q are NOT normalized on device: per-row ranking of sim_t is
    invariant to |t_b|, and the host corrects the sim_q payload by
    |q_b| (and computes exact dots for index-style candidates).
    Device preproc for t/q is just scale-cast + XBAR transpose + fp8.
  * grid coefficient 1024 (not 2048) so that an integer part up to 2047
    plus a 2^-13-granular index payload is exactly representable in
    fp32 (24-bit significand).
  * per [128,1024] psum pair, two styles (host replicates the rule):
      A  (3 of 4): ph1 DR -> ACT snap(+M, in psum) -> PE ones(-M)
                   -> ph2 DR (payload = FRS*|q|/sqrt(512)*sim_q) -> max8
      C2 (1 of 4): ph1 DR -> ACT snap(+M, psum->sbuf) ->
                   DVE stt((x-M)+iota*2^-13) -> max8
    A carries sim_q in the fraction; C2 carries the bank column index
    (host computes the exact dot for C2 winners).
  * engine budget/core (hot): PE ~205us, ACT ~165us, DVE ~230us target.
"""

import numpy as np

B, K, DIM, TOPK = 4096, 32768, 512, 5
NCORES = 8
KSH = K // NCORES  # 4096 bank rows per core

P = 128
CH = 512                      # psum-bank chunk width
PAIR = 2 * CH                 # 1024-wide psum pair (2 banks)
MAGIC = float(3 * (2 ** 22))  # fp32 add of +MAGIC snaps to integer grid
SQD = float(np.sqrt(DIM))     # 22.627
GRID = 1024.0                 # int part = round(GRID * (|t_b|/sqrt(D)) * sim_t)
TSC = 448.0                   # t scale: t_fp8 = t_raw * TSC/sqrt(D)
BSC = GRID / TSC              # bank scale (on normalized rows) = 2.2857
QSC = 0.5                     # q scale: q_fp8 = q_raw * QSC/sqrt(D)
FRS = QSC * BSC               # payload = FRS * (|q_b|/sqrt(D)) * sim_q
IDS = float(2.0 ** -13)       # index payload step
NCAND = 32                    # candidates per row per core (4 pairs x 8)

NB = B // P                   # 32 batch tiles
NPR = KSH // PAIR             # 4 pairs per batch tile
DCH = DIM // P                # 4 transpose chunks / 2 DR slice-pairs


def style_A(bt, pr):
    """True -> pair uses sim_q payload (ph2); False -> index payload.

    Exactly one C2 pair per batch tile, spread evenly in both pr-major
    (startup) and bt-major (steady) emission orders.
    """
    return pr != bt % 4


_CACHE = {}


def build_nc(b=B, ksh=KSH, dim=DIM, num_devices=NCORES):
    from contextlib import ExitStack

    import concourse.tile as tile
    from concourse import bacc, mybir
    from concourse.masks import make_identity

    f32 = mybir.dt.float32
    bf16 = mybir.dt.bfloat16
    fp8 = mybir.dt.float8e4
    Alu = mybir.AluOpType
    Act = mybir.ActivationFunctionType
    DR = mybir.MatmulPerfMode.DoubleRow

    NS = ksh // P             # 32 bank row-tiles
    TPP = NS // NPR           # 8 bank row-tiles per pair-column

    nc = bacc.Bacc(
        "TRN2", target_bir_lowering=False, debug=False, num_devices=num_devices
    )
    q_d = nc.dram_tensor("query_raw", [b, dim], f32, kind="ExternalInput").ap()
    t_d = nc.dram_tensor("target_raw", [b, dim], f32, kind="ExternalInput").ap()
    s_d = nc.dram_tensor("qshard", [ksh, dim], f32, kind="ExternalInput").ap()
    o_d = nc.dram_tensor("out", [b, NCAND], f32, kind="ExternalOutput").ap()

    MEG = 4                   # row-tiles per mega preproc group
    with tile.TileContext(nc) as tc, ExitStack() as ctx:
        singles = ctx.enter_context(tc.tile_pool(name="singles", bufs=1))
        ld = ctx.enter_context(tc.tile_pool(name="ld", bufs=2))
        ldb = ctx.enter_context(tc.tile_pool(name="ldb", bufs=2))
        xnp = ctx.enter_context(tc.tile_pool(name="xnp", bufs=3))
        xtp = ctx.enter_context(tc.tile_pool(name="xtp", bufs=2))
        nrm = ctx.enter_context(tc.tile_pool(name="nrm", bufs=6))
        small = ctx.enter_context(tc.tile_pool(name="small", bufs=8))
        psum = ctx.enter_context(tc.tile_pool(name="psum", bufs=4, space="PSUM"))
        c2p = ctx.enter_context(tc.tile_pool(name="c2p", bufs=3))

        onesc = singles.tile([1, P], bf16)
        mrow_n = singles.tile([1, CH], bf16)
        iota_raw = singles.tile([P, PAIR], f32)
        iotas = [singles.tile([P, PAIR], f32, name=f"iota{pr}")
                 for pr in range(NPR)]

        def emit_constants():
            nc.gpsimd.memset(onesc, 1.0)
            nc.gpsimd.memset(mrow_n, -MAGIC)
            # index payload tiles: iota_pr[j] = (pr*1024 + j) * 2^-13
            nc.gpsimd.iota(iota_raw, [[1, PAIR]], channel_multiplier=0,
                           allow_small_or_imprecise_dtypes=True)
            for pr in range(NPR):
                nc.vector.tensor_scalar(out=iotas[pr], in0=iota_raw,
                                        scalar1=IDS,
                                        scalar2=float(pr * PAIR) * IDS,
                                        op0=Alu.mult, op1=Alu.add)

        # Resident fp8 operands, DIM on partitions (DR slice layout).
        qbT = singles.tile([P, DCH, ksh], fp8)  # bank^T (normalized * BSC)
        tT = singles.tile([P, DCH, b], fp8)     # t_raw^T * TSC/sqrt(D)
        qT = singles.tile([P, DCH, b], fp8)     # q_raw^T * QSC/sqrt(D)

        cands = [singles.tile([P, NCAND], f32, name=f"cand{bt}")
                 for bt in range(NB)]

        # ---- t/q preproc (mega = 4 row-tiles = 512 rows per DMA/XBAR):
        # plain f32 mega load (sync HWDGE), per-tile scale-cast to bf16
        # (t on ACT, q on DVE), one mega XBAR each, per-tile fp8 SWDGE.
        def tq_load(m):
            dv = [P, MEG, dim]
            traw = ld.tile(dv, f32, tag="tr", name=f"tr{m}")
            src = t_d[m * MEG * P:(m + 1) * MEG * P, :]
            nc.gpsimd.dma_start(out=traw,
                                in_=src.rearrange("(r p) d -> p r d", p=P))
            qraw = ld.tile(dv, f32, tag="qr", name=f"qr{m}")
            src = q_d[m * MEG * P:(m + 1) * MEG * P, :]
            nc.gpsimd.dma_start(out=qraw,
                                in_=src.rearrange("(r p) d -> p r d", p=P))
            return traw, qraw

        def tq_cast(traw, qraw, m):
            tb = xnp.tile([P, MEG, dim], bf16, tag="tb", name=f"tb{m}")
            qb = xnp.tile([P, MEG, dim], bf16, tag="qb", name=f"qb{m}")
            for r in range(MEG):
                nc.scalar.activation(tb[:, r, :], traw[:, r, :], Act.Copy,
                                     scale=TSC / SQD)
            for r in range(MEG):
                nc.vector.tensor_scalar(out=qb[:, r, :], in0=qraw[:, r, :],
                                        scalar1=QSC / SQD, scalar2=None,
                                        op0=Alu.mult)
            return tb, qb

        def tq_xbar(tb, qb, m):
            xtt = xtp.tile([P, MEG * DCH, P], bf16, tag="xtt", name=f"txt{m}")
            nc.sync.dma_start_transpose(xtt, tb)
            xtq = xtp.tile([P, MEG * DCH, P], bf16, tag="xtq", name=f"qxt{m}")
            nc.sync.dma_start_transpose(xtq, qb)
            return xtt, xtq

        def tq_fp8(xtt, xtq, m):
            for r in range(MEG):
                it = m * MEG + r
                nc.gpsimd.dma_start(out=tT[:, :, it * P:(it + 1) * P],
                                    in_=xtt[:, r * DCH:(r + 1) * DCH, :])
            for r in range(MEG):
                it = m * MEG + r
                nc.gpsimd.dma_start(out=qT[:, :, it * P:(it + 1) * P],
                                    in_=xtq[:, r * DCH:(r + 1) * DCH, :])

        # ---- bank preproc stages (mega granularity).
        def bank_load(m):
            raw = ldb.tile([P, MEG, dim], f32, tag="raw", name=f"sr{m}")
            src = s_d[m * MEG * P:(m + 1) * MEG * P, :]
            nc.gpsimd.dma_start(out=raw,
                                in_=src.rearrange("(r p) d -> p r d", p=P))
            return raw

        def bank_norm(raw, m):
            """l2norm * BSC via ACT square-accum + DVE rsqrt + DVE cast."""
            xn = xnp.tile([P, MEG, dim], bf16, tag="xn", name=f"sxn{m}")
            for r in range(MEG):
                j = m * MEG + r
                ss = small.tile([P, 1], f32, tag="ss", name=f"sss{j}")
                sq = nrm.tile([P, dim], f32, tag="sq", name=f"ssq{j}")
                nc.scalar.activation(sq, raw[:, r, :], Act.Square, accum_out=ss)
                stdv = small.tile([P, 1], f32, tag="std", name=f"ssd{j}")
                nc.scalar.activation(stdv, ss, Act.Sqrt,
                                     scale=1.0 / (BSC * BSC))
                rin = small.tile([P, 1], f32, tag="rin", name=f"sri{j}")
                nc.vector.reciprocal(rin, stdv)
                nc.vector.tensor_scalar(out=xn[:, r, :], in0=raw[:, r, :],
                                        scalar1=rin, scalar2=None,
                                        op0=Alu.mult)
            return xn

        def bank_xbar(xn, m):
            xt = xtp.tile([P, MEG * DCH, P], bf16, tag="xt", name=f"sxt{m}")
            nc.sync.dma_start_transpose(xt, xn)
            for r in range(MEG):
                j = m * MEG + r
                nc.gpsimd.dma_start(out=qbT[:, :, j * P:(j + 1) * P],
                                    in_=xt[:, r * DCH:(r + 1) * DCH, :])

        def ph1(bt, pr, close_group=False):
            bs = slice(bt * P, (bt + 1) * P)
            pv = psum.tile([P, PAIR], f32, tag="pv", name=f"pv{bt}_{pr}")
            for c in range(2):
                ks = slice((pr * 2 + c) * CH, (pr * 2 + c + 1) * CH)
                for dr in range(2):
                    nc.tensor.matmul(pv[:, c * CH:(c + 1) * CH],
                                     tT[:, 2 * dr:2 * dr + 2, bs],
                                     qbT[:, 2 * dr:2 * dr + 2, ks],
                                     start=(dr == 0),
                                     stop=(close_group and dr == 1),
                                     perf_mode=DR)
            return pv

        def snap_A(pv):
            nc.scalar.activation(pv, pv, Act.Copy, bias=MAGIC)

        def finish_A(bt, pr, pv):
            """ones(-M) + ph2 payload matmuls + max8 from psum."""
            bs = slice(bt * P, (bt + 1) * P)
            for c in range(2):
                nc.tensor.matmul(pv[:, c * CH:(c + 1) * CH], onesc, mrow_n,
                                 start=False, stop=False, skip_group_check=True)
            for c in range(2):
                ks = slice((pr * 2 + c) * CH, (pr * 2 + c + 1) * CH)
                for dr in range(2):
                    nc.tensor.matmul(pv[:, c * CH:(c + 1) * CH],
                                     qT[:, 2 * dr:2 * dr + 2, bs],
                                     qbT[:, 2 * dr:2 * dr + 2, ks],
                                     start=False, stop=(dr == 1), perf_mode=DR)
            nc.vector.max(cands[bt][:, pr * 8:(pr + 1) * 8], pv)

        def finish_C2(bt, pr, pv):
            """snap-evac to sbuf, -M + iota on DVE, max8 from sbuf."""
            ev = c2p.tile([P, PAIR], f32, tag="ev", name=f"ev{bt}_{pr}")
            nc.scalar.activation(ev, pv, Act.Copy, bias=MAGIC)
            nc.vector.scalar_tensor_tensor(out=ev, in0=ev, scalar=-MAGIC,
                                           in1=iotas[pr], op0=Alu.add,
                                           op1=Alu.add)
            nc.vector.max(cands[bt][:, pr * 8:(pr + 1) * 8], ev)

        # ---------------- emission ----------------
        # Startup is slice-major over the first SB batch tiles: as soon as
        # bank mega 0/1 are resident, the PE has SB pairs of work while
        # the remaining megas and t/q stream in.  After that, bt-major.
        #
        # Preproc stages are emitted with an item LAG between dependent
        # stages so queue-head waits are (nearly) always pre-satisfied.
        PF = 2                     # tq lookahead, in megas
        SB = min(12, NB)
        LAG = 2
        items = [(bt, pr) for pr in range(NPR) for bt in range(SB)] + \
                [(bt, pr) for bt in range(SB, NB) for pr in range(NPR)]

        from collections import deque
        stq2 = deque()  # tq cast:  (emit_item, (traw, qraw, m))
        stq3 = deque()  # tq xbar:  (emit_item, (tb, qb, m))
        stq4 = deque()  # tq fp8:   (emit_item, (xtt, xtq, m))
        sb2 = deque()   # bank norm: (emit_item, (raw, m))
        sb3 = deque()   # bank xbar: (emit_item, (xn, m))

        def trig_tq(i, m):
            stq2.append((i, tq_load(m) + (m,)))

        def trig_bank(i, m):
            sb2.append((i, (bank_load(m), m)))

        def drain_pre(i, force=False):
            while sb2 and (force or sb2[0][0] <= i - LAG):
                ei, (raw, m) = sb2.popleft()
                sb3.append((ei + LAG, (bank_norm(raw, m), m)))
            while sb3 and (force or sb3[0][0] <= i - LAG):
                _, pl = sb3.popleft()
                bank_xbar(*pl)
            while stq2 and (force or stq2[0][0] <= i - LAG):
                ei, (a, b_, m) = stq2.popleft()
                stq3.append((ei + LAG, tq_cast(a, b_, m) + (m,)))
            while stq3 and (force or stq3[0][0] <= i - LAG):
                ei, (a, b_, m) = stq3.popleft()
                stq4.append((ei + LAG, tq_xbar(a, b_, m) + (m,)))
            while stq4 and (force or stq4[0][0] <= i - LAG):
                _, pl = stq4.popleft()
                tq_fp8(*pl)

        pend = None  # (bt, pr, pv) awaiting finish_A
        for i, (bt, pr) in enumerate(items):
            startup = i < SB * NPR
            if i == 0:
                # queue the loads the first phase needs, then run the
                # dependent stages immediately (loads already queued).
                trig_bank(i, 0)
                trig_bank(i, 1)
                trig_tq(i, 0)
                emit_constants()
                drain_pre(i, force=True)
            if startup:
                # tq megas 1,2 stream early in pr 0; bank megas for phase
                # pr+1 during phase pr; tq megas 3,4 late in startup.
                if pr == 0 and bt in (2, 6):
                    trig_tq(i, 1 + (bt - 2) // 4)
                if pr + 1 < NPR and bt in (0, 2):
                    trig_bank(i, 2 * (pr + 1) + bt // 2)
                if pr == NPR - 1 and bt in (0, 4):
                    trig_tq(i, 3 + bt // 4)
            else:
                # steady: trigger tq mega (bt//4 + PF) at mega boundaries
                if pr == 0 and bt % MEG == 0 and (bt // MEG + PF) * MEG < NB:
                    trig_tq(i, bt // MEG + PF)
            drain_pre(i)

            isA = style_A(bt, pr)
            pv = ph1(bt, pr, close_group=not isA)
            if isA:
                snap_A(pv)
                if pend is not None:
                    finish_A(*pend)
                pend = (bt, pr, pv)
            else:
                if pend is not None:
                    finish_A(*pend)
                    pend = None
                finish_C2(bt, pr, pv)
        drain_pre(len(items), force=True)
        if pend is not None:
            finish_A(*pend)
        for bt in range(NB):
            nc.gpsimd.dma_start(out=o_d[bt * P:(bt + 1) * P, :], in_=cands[bt])

    nc.compile()
    return nc


def _get_nc():
    key = (B, KSH, DIM, NCORES)
    if key not in _CACHE:
        _CACHE[key] = build_nc()
    return _CACHE[key]


def merge_host(cand_v, query_raw, target_raw, queue, topk=TOPK):
    """cand_v: [ncores, b, NCAND] packed values -> scalar loss.

    Per candidate slot s of core c for row r:
      pr = s // 8, bt = r // 128.
      v = int + frac, int = round(GRID * rho_t * sim_t).
      style A : frac = FRS * (|q_r|/sqrt(D)) * sim_q   (frac in (-.5,.5))
      style C2: frac = (pr*1024 + j) * 2^-13, bank col = c*KSH + pr*1024+j
    """
    nc_, b, ncand = cand_v.shape
    q = np.asarray(query_raw, dtype=np.float64)
    t = np.asarray(target_raw, dtype=np.float64)
    qu = np.asarray(queue, dtype=np.float64)
    qn = np.linalg.norm(q, axis=1)                      # |q_r|

    v = np.transpose(cand_v.astype(np.float64), (1, 0, 2))  # [b, nc, NCAND]
    v = v.reshape(b, nc_ * ncand)
    vint = np.round(v)
    frac = v - vint

    # style mask per (row, flat candidate slot): A iff pr != bt % 4
    slot = np.arange(nc_ * ncand) % ncand
    pr_of_slot = slot // 8                               # [nc*NCAND]
    bt_of_row = (np.arange(b) // P)                      # [b]
    isA = pr_of_slot[None, :] != (bt_of_row[:, None] % 4)

    # top-5 by packed value (ranking == int ranking up to grid ties)
    top_idx = np.argpartition(-v, topk - 1, axis=1)[:, :topk]   # [b, 5]
    rows = np.arange(b)[:, None]
    w_frac = frac[rows, top_idx]
    w_isA = isA[rows, top_idx]
    w_core = (top_idx // ncand)
    w_pr = (top_idx % ncand) // 8

    # style A: sim_q from payload
    sim_q = np.zeros((b, topk))
    coefA = FRS * (qn / SQD)                             # [b]
    sim_q = np.where(w_isA, w_frac / coefA[:, None], 0.0)

    # style C2: exact dot for winners
    c2_rows, c2_cols = np.nonzero(~w_isA)
    if c2_rows.size:
        j_local = np.rint(w_frac[c2_rows, c2_cols] / IDS).astype(np.int64)
        g = w_core[c2_rows, c2_cols] * KSH + j_local     # global bank row
        bank_rows = np.where((g < B)[:, None],
                             t[np.minimum(g, B - 1)],
                             qu[np.minimum(g, K - 1)])
        bank_rows = bank_rows / np.linalg.norm(bank_rows, axis=1, keepdims=True)
        qrows = q[c2_rows] / qn[c2_rows][:, None]
        sim_q[c2_rows, c2_cols] = np.einsum('ij,ij->i', qrows, bank_rows)

    dist_q = 2.0 - 2.0 * sim_q
    return np.float32(dist_q.mean())


def run_device(query_raw, target_raw, queue, **spmd_kwargs):
    from concourse.bass_utils import run_bass_kernel_spmd

    q = np.ascontiguousarray(np.asarray(query_raw, dtype=np.float32))
    t = np.ascontiguousarray(np.asarray(target_raw, dtype=np.float32))
    qu = np.ascontiguousarray(np.asarray(queue, dtype=np.float32))

    nc = _get_nc()
    in_maps = []
    for c in range(NCORES):
        shard = t if c == 0 else qu[c * KSH:(c + 1) * KSH]
        in_maps.append(
            {"query_raw": q, "target_raw": t,
             "qshard": np.ascontiguousarray(shard)}
        )
    bres = run_bass_kernel_spmd(nc, in_maps, list(range(NCORES)), **spmd_kwargs)
    cand = np.stack([bres.results[c]["out"] for c in range(NCORES)], axis=0)
    return merge_host(cand, q, t, qu), bres


def kernel(query_raw, target_raw, queue):
    loss, _ = run_device(query_raw, target_raw, queue)
    return loss


# revision 32
# speedup vs baseline: 1.0104x; 1.0104x over previous
"""MeanShift retrieval-KNN loss kernel for 8 Trainium2 NeuronCores — v3.

Reference computation (B=4096, K=32768, DIM=512, TOPK=5):
    query  = l2norm(query_raw); target = l2norm(target_raw)
    qbank  = l2norm(queue); qbank[0:B] = target
    dist_t = 2 - 2 * target @ qbank.T ; dist_q = 2 - 2 * query @ qbank.T
    idx    = top5 smallest dist_t per row
    loss   = mean_b( sum_j dist_q[b, idx[b,j]] / 5 )

Sharding: queue K axis split across 8 cores (4096 rows each); core 0's
shard is target_raw (the reference overwrites bank rows 0:B, and raw
queue rows 0:B are never read).

v3 design (vs v2):
  * t/q are NOT normalized on device: per-row ranking of sim_t is
    invariant to |t_b|, and the host corrects the sim_q payload by
    |q_b| (and computes exact dots for index-style candidates).
    Device preproc for t/q is just scale-cast + XBAR transpose + fp8.
  * grid coefficient 1024 (not 2048) so that an integer part up to 2047
    plus a 2^-13-granular index payload is exactly representable in
    fp32 (24-bit significand).
  * per [128,1024] psum pair, two styles (host replicates the rule):
      A  (3 of 4): ph1 DR -> ACT snap(+M, in psum) -> PE ones(-M)
                   -> ph2 DR (payload = FRS*|q|/sqrt(512)*sim_q) -> max8
      C2 (1 of 4): ph1 DR -> ACT snap(+M, psum->sbuf) ->
                   DVE stt((x-M)+iota*2^-13) -> max8
    A carries sim_q in the fraction; C2 carries the bank column index
    (host computes the exact dot for C2 winners).
  * engine budget/core (hot): PE ~205us, ACT ~165us, DVE ~230us target.
"""

import numpy as np

B, K, DIM, TOPK = 4096, 32768, 512, 5
NCORES = 8
KSH = K // NCORES  # 4096 bank rows per core

P = 128
CH = 512                      # psum-bank chunk width
PAIR = 2 * CH                 # 1024-wide psum pair (2 banks)
MAGIC = float(3 * (2 ** 22))  # fp32 add of +MAGIC snaps to integer grid
SQD = float(np.sqrt(DIM))     # 22.627
GRID = 1024.0                 # int part = round(GRID * (|t_b|/sqrt(D)) * sim_t)
TSC = 448.0                   # t scale: t_fp8 = t_raw * TSC/sqrt(D)
BSC = GRID / TSC              # bank scale (on normalized rows) = 2.2857
QSC = 0.5                     # q scale: q_fp8 = q_raw * QSC/sqrt(D)
FRS = QSC * BSC               # payload = FRS * (|q_b|/sqrt(D)) * sim_q
IDS = float(2.0 ** -13)       # index payload step
NCAND = 32                    # candidates per row per core (4 pairs x 8)

NB = B // P                   # 32 batch tiles
NPR = KSH // PAIR             # 4 pairs per batch tile
DCH = DIM // P                # 4 transpose chunks / 2 DR slice-pairs


def style_A(bt, pr):
    """True -> pair uses sim_q payload (ph2); False -> index payload.

    Exactly one C2 pair per batch tile, spread evenly in both pr-major
    (startup) and bt-major (steady) emission orders.
    """
    return pr != bt % 4


_CACHE = {}


def build_nc(b=B, ksh=KSH, dim=DIM, num_devices=NCORES):
    from contextlib import ExitStack

    import concourse.tile as tile
    from concourse import bacc, mybir
    from concourse.masks import make_identity

    f32 = mybir.dt.float32
    bf16 = mybir.dt.bfloat16
    fp8 = mybir.dt.float8e4
    Alu = mybir.AluOpType
    Act = mybir.ActivationFunctionType
    DR = mybir.MatmulPerfMode.DoubleRow

    NS = ksh // P             # 32 bank row-tiles
    TPP = NS // NPR           # 8 bank row-tiles per pair-column

    nc = bacc.Bacc(
        "TRN2", target_bir_lowering=False, debug=False, num_devices=num_devices
    )
    q_d = nc.dram_tensor("query_raw", [b, dim], f32, kind="ExternalInput").ap()
    t_d = nc.dram_tensor("target_raw", [b, dim], f32, kind="ExternalInput").ap()
    s_d = nc.dram_tensor("qshard", [ksh, dim], f32, kind="ExternalInput").ap()
    o_d = nc.dram_tensor("out", [b, NCAND], f32, kind="ExternalOutput").ap()

    MEG = 4                   # row-tiles per mega preproc group
    with tile.TileContext(nc) as tc, ExitStack() as ctx:
        singles = ctx.enter_context(tc.tile_pool(name="singles", bufs=1))
        ld = ctx.enter_context(tc.tile_pool(name="ld", bufs=2))
        ldb = ctx.enter_context(tc.tile_pool(name="ldb", bufs=2))
        xnp = ctx.enter_context(tc.tile_pool(name="xnp", bufs=3))
        xtp = ctx.enter_context(tc.tile_pool(name="xtp", bufs=2))
        nrm = ctx.enter_context(tc.tile_pool(name="nrm", bufs=6))
        small = ctx.enter_context(tc.tile_pool(name="small", bufs=8))
        psum = ctx.enter_context(tc.tile_pool(name="psum", bufs=4, space="PSUM"))
        c2p = ctx.enter_context(tc.tile_pool(name="c2p", bufs=3))
        tok = ctx.enter_context(tc.tile_pool(name="tok", bufs=2))

        onesc = singles.tile([1, P], bf16)
        mrow_n = singles.tile([1, CH], bf16)
        iota_raw = singles.tile([P, PAIR], f32)
        iotas = [singles.tile([P, PAIR], f32, name=f"iota{pr}")
                 for pr in range(NPR)]

        def emit_constants():
            nc.gpsimd.memset(onesc, 1.0)
            nc.gpsimd.memset(mrow_n, -MAGIC)
            # index payload tiles: iota_pr[j] = (pr*1024 + j) * 2^-13
            nc.gpsimd.iota(iota_raw, [[1, PAIR]], channel_multiplier=0,
                           allow_small_or_imprecise_dtypes=True)
            for pr in range(NPR):
                nc.vector.tensor_scalar(out=iotas[pr], in0=iota_raw,
                                        scalar1=IDS,
                                        scalar2=float(pr * PAIR) * IDS,
                                        op0=Alu.mult, op1=Alu.add)

        # Resident fp8 operands, DIM on partitions (DR slice layout).
        qbT = singles.tile([P, DCH, ksh], fp8)  # bank^T (normalized * BSC)
        tT = singles.tile([P, DCH, b], fp8)     # t_raw^T * TSC/sqrt(D)
        qT = singles.tile([P, DCH, b], fp8)     # q_raw^T * QSC/sqrt(D)

        cands = [singles.tile([P, NCAND], f32, name=f"cand{bt}")
                 for bt in range(NB)]

        # ---- t/q preproc (mega = 4 row-tiles = 512 rows per DMA/XBAR):
        # plain f32 mega load (sync HWDGE), per-tile scale-cast to bf16
        # (t on ACT, q on DVE), one mega XBAR each, per-tile fp8 SWDGE.
        def tq_load(m):
            dv = [P, MEG, dim]
            traw = ld.tile(dv, f32, tag="tr", name=f"tr{m}")
            src = t_d[m * MEG * P:(m + 1) * MEG * P, :]
            nc.sync.dma_start(out=traw,
                              in_=src.rearrange("(r p) d -> p r d", p=P))
            qraw = ld.tile(dv, f32, tag="qr", name=f"qr{m}")
            src = q_d[m * MEG * P:(m + 1) * MEG * P, :]
            nc.sync.dma_start(out=qraw,
                              in_=src.rearrange("(r p) d -> p r d", p=P))
            return traw, qraw

        tok_ring = {'t': [None, None], 'q': [None, None], 's': [None, None]}

        def tq_cast(traw, qraw, m):
            tb = xnp.tile([P, MEG, dim], bf16, tag="tb", name=f"tb{m}")
            qb = xnp.tile([P, MEG, dim], bf16, tag="qb", name=f"qb{m}")
            # tokens carry a data dep from the fp8 DMAs that last READ the
            # xt ring slot this mega's xbar will overwrite (DMA-read ->
            # DMA-write WAR is not tracked by the framework; engine ops are).
            tkt = tok_ring['t'][m % 2]
            tsc = TSC / SQD if tkt is None else tkt[:, 0:1]
            tkq = tok_ring['q'][m % 2]
            for r in range(MEG):
                nc.scalar.activation(tb[:, r, :], traw[:, r, :], Act.Copy,
                                     scale=tsc)
            for r in range(MEG):
                if tkq is None:
                    nc.vector.tensor_scalar(out=qb[:, r, :],
                                            in0=qraw[:, r, :],
                                            scalar1=QSC / SQD, scalar2=None,
                                            op0=Alu.mult)
                else:
                    nc.vector.tensor_scalar(out=qb[:, r, :],
                                            in0=qraw[:, r, :],
                                            scalar1=QSC / SQD,
                                            scalar2=tkq[:, 0:1],
                                            op0=Alu.mult, op1=Alu.bypass)
            return tb, qb

        def tq_xbar(tb, qb, m):
            xtt = xtp.tile([P, MEG * DCH, P], bf16, tag="xtt", name=f"txt{m}")
            nc.sync.dma_start_transpose(xtt, tb)
            xtq = xtp.tile([P, MEG * DCH, P], bf16, tag="xtq", name=f"qxt{m}")
            nc.sync.dma_start_transpose(xtq, qb)
            return xtt, xtq

        def tq_fp8(xtt, xtq, m):
            for r in range(MEG):
                it = m * MEG + r
                nc.gpsimd.dma_start(out=tT[:, :, it * P:(it + 1) * P],
                                    in_=xtt[:, r * DCH:(r + 1) * DCH, :])
            for r in range(MEG):
                it = m * MEG + r
                nc.gpsimd.dma_start(out=qT[:, :, it * P:(it + 1) * P],
                                    in_=xtq[:, r * DCH:(r + 1) * DCH, :])
            tkt = tok.tile([P, MEG], f32, tag="tokT", name=f"tokT{m}")
            nc.vector.tensor_scalar(
                out=tkt, in0=tT[:, 0, m * MEG * P:(m + 1) * MEG * P:P],
                scalar1=0.0, scalar2=TSC / SQD, op0=Alu.mult, op1=Alu.add)
            tok_ring['t'][m % 2] = tkt
            tkq = tok.tile([P, MEG], f32, tag="tokQ", name=f"tokQ{m}")
            nc.vector.tensor_scalar(
                out=tkq, in0=qT[:, 0, m * MEG * P:(m + 1) * MEG * P:P],
                scalar1=0.0, scalar2=1.0, op0=Alu.mult, op1=Alu.add)
            tok_ring['q'][m % 2] = tkq

        # ---- bank preproc stages (mega granularity).
        def bank_load(m):
            raw = ldb.tile([P, MEG, dim], f32, tag="raw", name=f"sr{m}")
            src = s_d[m * MEG * P:(m + 1) * MEG * P, :]
            nc.sync.dma_start(out=raw,
                              in_=src.rearrange("(r p) d -> p r d", p=P))
            return raw

        def bank_norm(raw, m):
            """l2norm * BSC via ACT square-accum + DVE rsqrt + DVE cast."""
            xn = xnp.tile([P, MEG, dim], bf16, tag="xn", name=f"sxn{m}")
            for r in range(MEG):
                j = m * MEG + r
                ss = small.tile([P, 1], f32, tag="ss", name=f"sss{j}")
                sq = nrm.tile([P, dim], f32, tag="sq", name=f"ssq{j}")
                nc.scalar.activation(sq, raw[:, r, :], Act.Square, accum_out=ss)
                stdv = small.tile([P, 1], f32, tag="std", name=f"ssd{j}")
                nc.scalar.activation(stdv, ss, Act.Sqrt,
                                     scale=1.0 / (BSC * BSC))
                rin = small.tile([P, 1], f32, tag="rin", name=f"sri{j}")
                nc.vector.reciprocal(rin, stdv)
                tks = tok_ring['s'][m % 2]
                if tks is None:
                    nc.vector.tensor_scalar(out=xn[:, r, :], in0=raw[:, r, :],
                                            scalar1=rin, scalar2=None,
                                            op0=Alu.mult)
                else:
                    nc.vector.tensor_scalar(out=xn[:, r, :], in0=raw[:, r, :],
                                            scalar1=rin, scalar2=tks[:, 0:1],
                                            op0=Alu.mult, op1=Alu.bypass)
            return xn

        def bank_xbar(xn, m):
            xt = xtp.tile([P, MEG * DCH, P], bf16, tag="xt", name=f"sxt{m}")
            nc.sync.dma_start_transpose(xt, xn)
            for r in range(MEG):
                j = m * MEG + r
                nc.gpsimd.dma_start(out=qbT[:, :, j * P:(j + 1) * P],
                                    in_=xt[:, r * DCH:(r + 1) * DCH, :])
            tks = tok.tile([P, MEG], f32, tag="tokS", name=f"tokS{m}")
            nc.vector.tensor_scalar(
                out=tks, in0=qbT[:, 0, m * MEG * P:(m + 1) * MEG * P:P],
                scalar1=0.0, scalar2=1.0, op0=Alu.mult, op1=Alu.add)
            tok_ring['s'][m % 2] = tks

        def ph1(bt, pr, close_group=False):
            bs = slice(bt * P, (bt + 1) * P)
            pv = psum.tile([P, PAIR], f32, tag="pv", name=f"pv{bt}_{pr}")
            for c in range(2):
                ks = slice((pr * 2 + c) * CH, (pr * 2 + c + 1) * CH)
                for dr in range(2):
                    nc.tensor.matmul(pv[:, c * CH:(c + 1) * CH],
                                     tT[:, 2 * dr:2 * dr + 2, bs],
                                     qbT[:, 2 * dr:2 * dr + 2, ks],
                                     start=(dr == 0),
                                     stop=(close_group and dr == 1),
                                     perf_mode=DR)
            return pv

        def snap_A(pv):
            nc.scalar.activation(pv, pv, Act.Copy, bias=MAGIC)

        def finish_A(bt, pr, pv):
            """ones(-M) + ph2 payload matmuls + max8 from psum."""
            bs = slice(bt * P, (bt + 1) * P)
            for c in range(2):
                nc.tensor.matmul(pv[:, c * CH:(c + 1) * CH], onesc, mrow_n,
                                 start=False, stop=False, skip_group_check=True)
            for c in range(2):
                ks = slice((pr * 2 + c) * CH, (pr * 2 + c + 1) * CH)
                for dr in range(2):
                    nc.tensor.matmul(pv[:, c * CH:(c + 1) * CH],
                                     qT[:, 2 * dr:2 * dr + 2, bs],
                                     qbT[:, 2 * dr:2 * dr + 2, ks],
                                     start=False, stop=(dr == 1), perf_mode=DR)
            nc.vector.max(cands[bt][:, pr * 8:(pr + 1) * 8], pv)

        def finish_C2(bt, pr, pv):
            """snap-evac to sbuf, -M + iota on DVE, max8 from sbuf."""
            ev = c2p.tile([P, PAIR], f32, tag="ev", name=f"ev{bt}_{pr}")
            nc.scalar.activation(ev, pv, Act.Copy, bias=MAGIC)
            nc.vector.scalar_tensor_tensor(out=ev, in0=ev, scalar=-MAGIC,
                                           in1=iotas[pr], op0=Alu.add,
                                           op1=Alu.add)
            nc.vector.max(cands[bt][:, pr * 8:(pr + 1) * 8], ev)

        # ---------------- emission ----------------
        # Startup is slice-major over the first SB batch tiles: as soon as
        # bank mega 0/1 are resident, the PE has SB pairs of work while
        # the remaining megas and t/q stream in.  After that, bt-major.
        #
        # Preproc stages are emitted with an item LAG between dependent
        # stages so queue-head waits are (nearly) always pre-satisfied.
        PF = 2                     # tq lookahead, in megas
        SB = min(12, NB)
        LAG = 2
        items = [(bt, pr) for pr in range(NPR) for bt in range(SB)] + \
                [(bt, pr) for bt in range(SB, NB) for pr in range(NPR)]

        from collections import deque
        stq2 = deque()  # tq cast:  (emit_item, (traw, qraw, m))
        stq3 = deque()  # tq xbar:  (emit_item, (tb, qb, m))
        stq4 = deque()  # tq fp8:   (emit_item, (xtt, xtq, m))
        sb2 = deque()   # bank norm: (emit_item, (raw, m))
        sb3 = deque()   # bank xbar: (emit_item, (xn, m))

        def trig_tq(i, m):
            stq2.append((i, tq_load(m) + (m,)))

        def trig_bank(i, m):
            sb2.append((i, (bank_load(m), m)))

        def drain_pre(i, force=False):
            while sb2 and (force or sb2[0][0] <= i - LAG):
                ei, (raw, m) = sb2.popleft()
                sb3.append((ei + LAG, (bank_norm(raw, m), m)))
            while sb3 and (force or sb3[0][0] <= i - LAG):
                _, pl = sb3.popleft()
                bank_xbar(*pl)
            while stq2 and (force or stq2[0][0] <= i - LAG):
                ei, (a, b_, m) = stq2.popleft()
                stq3.append((ei + LAG, tq_cast(a, b_, m) + (m,)))
            while stq3 and (force or stq3[0][0] <= i - LAG):
                ei, (a, b_, m) = stq3.popleft()
                stq4.append((ei + LAG, tq_xbar(a, b_, m) + (m,)))
            while stq4 and (force or stq4[0][0] <= i - LAG):
                _, pl = stq4.popleft()
                tq_fp8(*pl)

        pend = None  # (bt, pr, pv) awaiting finish_A
        for i, (bt, pr) in enumerate(items):
            startup = i < SB * NPR
            if i == 0:
                # queue the loads the first phase needs, then run the
                # dependent stages immediately (loads already queued).
                trig_bank(i, 0)
                trig_bank(i, 1)
                trig_tq(i, 0)
                emit_constants()
                drain_pre(i, force=True)
            if startup:
                # tq megas 1,2 stream early in pr 0; bank megas for phase
                # pr+1 during phase pr; tq megas 3,4 late in startup.
                if pr == 0 and bt in (2, 6):
                    trig_tq(i, 1 + (bt - 2) // 4)
                if pr + 1 < NPR and bt in (0, 2):
                    trig_bank(i, 2 * (pr + 1) + bt // 2)
                if pr == NPR - 1 and bt in (0, 4):
                    trig_tq(i, 3 + bt // 4)
            else:
                # steady: trigger tq mega (bt//4 + PF) at mega boundaries
                if pr == 0 and bt % MEG == 0 and (bt // MEG + PF) * MEG < NB:
                    trig_tq(i, bt // MEG + PF)
            drain_pre(i)

            isA = style_A(bt, pr)
            pv = ph1(bt, pr, close_group=not isA)
            if isA:
                snap_A(pv)
                if pend is not None:
                    finish_A(*pend)
                pend = (bt, pr, pv)
            else:
                if pend is not None:
                    finish_A(*pend)
                    pend = None
                finish_C2(bt, pr, pv)
        drain_pre(len(items), force=True)
        if pend is not None:
            finish_A(*pend)
        for bt in range(NB):
            nc.gpsimd.dma_start(out=o_d[bt * P:(bt + 1) * P, :], in_=cands[bt])

    nc.compile()
    return nc


def _get_nc():
    key = (B, KSH, DIM, NCORES)
    if key not in _CACHE:
        _CACHE[key] = build_nc()
    return _CACHE[key]


def merge_host(cand_v, query_raw, target_raw, queue, topk=TOPK):
    """cand_v: [ncores, b, NCAND] packed values -> scalar loss.

    Per candidate slot s of core c for row r:
      pr = s // 8, bt = r // 128.
      v = int + frac, int = round(GRID * rho_t * sim_t).
      style A : frac = FRS * (|q_r|/sqrt(D)) * sim_q   (frac in (-.5,.5))
      style C2: frac = (pr*1024 + j) * 2^-13, bank col = c*KSH + pr*1024+j
    """
    nc_, b, ncand = cand_v.shape
    q = np.asarray(query_raw, dtype=np.float64)
    t = np.asarray(target_raw, dtype=np.float64)
    qu = np.asarray(queue, dtype=np.float64)
    qn = np.linalg.norm(q, axis=1)                      # |q_r|

    v = np.transpose(cand_v.astype(np.float64), (1, 0, 2))  # [b, nc, NCAND]
    v = v.reshape(b, nc_ * ncand)
    vint = np.round(v)
    frac = v - vint

    # style mask per (row, flat candidate slot): A iff pr != bt % 4
    slot = np.arange(nc_ * ncand) % ncand
    pr_of_slot = slot // 8                               # [nc*NCAND]
    bt_of_row = (np.arange(b) // P)                      # [b]
    isA = pr_of_slot[None, :] != (bt_of_row[:, None] % 4)

    # top-5 by packed value (ranking == int ranking up to grid ties)
    top_idx = np.argpartition(-v, topk - 1, axis=1)[:, :topk]   # [b, 5]
    rows = np.arange(b)[:, None]
    w_frac = frac[rows, top_idx]
    w_isA = isA[rows, top_idx]
    w_core = (top_idx // ncand)
    w_pr = (top_idx % ncand) // 8

    # style A: sim_q from payload
    sim_q = np.zeros((b, topk))
    coefA = FRS * (qn / SQD)                             # [b]
    sim_q = np.where(w_isA, w_frac / coefA[:, None], 0.0)

    # style C2: exact dot for winners
    c2_rows, c2_cols = np.nonzero(~w_isA)
    if c2_rows.size:
        j_local = np.rint(w_frac[c2_rows, c2_cols] / IDS).astype(np.int64)
        g = w_core[c2_rows, c2_cols] * KSH + j_local     # global bank row
        bank_rows = np.where((g < B)[:, None],
                             t[np.minimum(g, B - 1)],
                             qu[np.minimum(g, K - 1)])
        bank_rows = bank_rows / np.linalg.norm(bank_rows, axis=1, keepdims=True)
        qrows = q[c2_rows] / qn[c2_rows][:, None]
        sim_q[c2_rows, c2_cols] = np.einsum('ij,ij->i', qrows, bank_rows)

    dist_q = 2.0 - 2.0 * sim_q
    return np.float32(dist_q.mean())


def run_device(query_raw, target_raw, queue, **spmd_kwargs):
    from concourse.bass_utils import run_bass_kernel_spmd

    q = np.ascontiguousarray(np.asarray(query_raw, dtype=np.float32))
    t = np.ascontiguousarray(np.asarray(target_raw, dtype=np.float32))
    qu = np.ascontiguousarray(np.asarray(queue, dtype=np.float32))

    nc = _get_nc()
    in_maps = []
    for c in range(NCORES):
        shard = t if c == 0 else qu[c * KSH:(c + 1) * KSH]
        in_maps.append(
            {"query_raw": q, "target_raw": t,
             "qshard": np.ascontiguousarray(shard)}
        )
    bres = run_bass_kernel_spmd(nc, in_maps, list(range(NCORES)), **spmd_kwargs)
    cand = np.stack([bres.results[c]["out"] for c in range(NCORES)], axis=0)
    return merge_host(cand, q, t, qu), bres


def kernel(query_raw, target_raw, queue):
    loss, _ = run_device(query_raw, target_raw, queue)
    return loss
